# revision 1
# baseline (speedup 1.0000x reference)
"""DCNv3 forward on 8 trn2 NeuronCores.

Strategy (data-parallel over (batch, H-half) -> 8 shards):
  - host: pre-transpose per-shard input into the layouts the device wants
    (zero-padded pixel slab for sampling; CHW tile for the offset/mask matmuls)
  - device per core, per 4-row tile:
      PE matmul  : offsets (72) + mask logits (36) for 512 pixels
      PE transp  : move results to pixel-on-partition layout
      ACT        : exp, hat weights  relu(1 - |u - i|)
      DVE        : softmax-normalize, per-cell coefficients m*hy_i*hx_j
      DVE apply  : acc[g-slice] (+)= coef[(g,p,i,j)][wo] * Xshift[wo, c-slice]
                   (fused scalar_tensor_tensor, per-partition scalar)
  - bilinear gather is made gather-free: integer part of every sampling
    offset is bounded (|off|<=~2px), so sampling = sum over a per-(g,p)
    3-4 tap window of hat-weighted *fixed* shifts of the input, and every
    fixed shift is just an access-pattern offset into an SBUF slab.
"""

import numpy as np
import sys

sys.path.insert(0, "/opt/trn_rl_repo")

import concourse.bass as bass
import concourse.bacc as bacc
import concourse.mybir as mybir
import concourse.tile as tile
from concourse.bass_utils import run_bass_kernel_spmd

B, C, H, W = 4, 128, 128, 128
G, P, gc = 4, 9, 32
N_CORES = 8
HS = H // 2          # rows per core (b, half)
RT = 4               # output rows per device tile
NTILES = HS // RT    # 16
NTAP = 4             # hat taps per axis (span is 3 or 4 per (g,p))

f32 = mybir.dt.float32

_KS = np.array([-1.0, 0.0, 1.0], np.float32)
KX = np.repeat(_KS, 3)   # x-major flatten (matches torch meshgrid in ref)
KY = np.tile(_KS, 3)


def _geometry(inp, W_off, b_off):
    """Per-(g,p) integer tap bases/spans from the actual offset field."""
    xhw = inp.reshape(B, H, W, C)
    off = (xhw.reshape(-1, C) @ W_off + b_off).reshape(-1, G, P, 2)
    rx = off[..., 0] + KX          # offset (x) relative to wo+1  (padded coords)
    ry = off[..., 1] + KY
    Bx = np.floor(rx.min(axis=0)).astype(np.int64)
    By = np.floor(ry.min(axis=0)).astype(np.int64)
    spx = np.floor(rx.max(axis=0)).astype(np.int64) + 2 - Bx
    spy = np.floor(ry.max(axis=0)).astype(np.int64) + 2 - By
    spx = np.minimum(spx, NTAP)
    spy = np.minimum(spy, NTAP)
    assert spx.max() <= NTAP and spy.max() <= NTAP
    return Bx, By, spx, spy


class _Geom:
    pass


def _build(g: "_Geom"):
    nc = bacc.Bacc("TRN2", target_bir_lowering=False, debug=False,
                   num_devices=N_CORES)

    xslab_t = nc.dram_tensor("xslab", [g.NROW * g.NCOL * C], f32, kind="ExternalInput")
    xchw_t = nc.dram_tensor("xchw", [C, HS * W], f32, kind="ExternalInput")
    wcat_t = nc.dram_tensor("wcat", [C, 108], f32, kind="ExternalInput")
    addc_t = nc.dram_tensor("addc", [C, 108], f32, kind="ExternalInput")
    ident_t = nc.dram_tensor("ident", [C, C], f32, kind="ExternalInput")
    cvals_t = nc.dram_tensor("cvals", [C, 8], f32, kind="ExternalInput")
    out_t = nc.dram_tensor("out", [HS * W * C], f32, kind="ExternalOutput")

    NS, NR = g.NS, g.NR
    mult, add = mybir.AluOpType.mult, mybir.AluOpType.add
    AF = mybir.ActivationFunctionType

    def vap(v, off, dims):
        return bass.AP(tensor=v.tensor, offset=v.offset + off, ap=[v.ap[0]] + dims)

    with tile.TileContext(nc) as tc:
        with (
            tc.tile_pool(name="const", bufs=1) as cpool,
            tc.tile_pool(name="xs", bufs=2) as xspool,
            tc.tile_pool(name="work", bufs=2) as wpool,
            tc.tile_pool(name="psum", bufs=2, space="PSUM") as pspool,
        ):
            wcat0 = cpool.tile([C, 108], f32)
            wcat = cpool.tile([C, 108], f32)
            addc = cpool.tile([C, 108], f32)
            ident = cpool.tile([C, C], f32)
            cvals = cpool.tile([C, 8], f32)
            nc.sync.dma_start(wcat0[:], wcat_t.ap())
            nc.sync.dma_start(addc[:], addc_t.ap())
            nc.sync.dma_start(ident[:], ident_t.ap())
            nc.sync.dma_start(cvals[:], cvals_t.ap())
            # matmul operands come via ACT copies: the Matmult HW struct has a
            # single sync-wait slot, so all its deps must arrive on one sem
            nc.scalar.copy(wcat[:], wcat0[:])

            for t in range(NTILES):
                # ---- loads -------------------------------------------------
                xs = xspool.tile([C, NS * NR * C], f32, name="xs")
                for si in range(NS):
                    src = bass.AP(
                        tensor=xslab_t,
                        offset=(RT * t * g.NCOL + g.C0 + si) * C,
                        ap=[[C, W], [g.NCOL * C, NR], [1, C]])
                    nc.sync.dma_start(
                        vap(xs[:], si * NR * C, [[C, NR], [1, C]]), src)

                xc0 = wpool.tile([C, RT * W], f32, name="xc0")
                nc.sync.dma_start(
                    xc0[:], bass.AP(tensor=xchw_t, offset=RT * t * W,
                                    ap=[[HS * W, C], [1, RT * W]]))
                xc = wpool.tile([C, RT * W], f32, name="xc")
                nc.scalar.copy(xc[:], xc0[:])

                # ---- offsets / logits (PE), directly in q-on-partitions ----
                rawq = wpool.tile([C, RT * 108], f32, name="rawq")
                for k in range(RT):
                    praw = pspool.tile([C, 108], f32, name="praw")
                    nc.tensor.matmul(praw[:], xc[:, k * W:(k + 1) * W],
                                     wcat[:], start=True, stop=True)
                    nc.scalar.copy(vap(rawq[:], k * 108, [[1, 108]]), praw[:])

                # rawq[:, r*108 + k] : k 0..35 y-offs, 36..71 x-offs, 72..107 logits
                nc.vector.tensor_tensor(
                    vap(rawq[:], 0, [[108, RT], [1, 108]]),
                    vap(rawq[:], 0, [[108, RT], [1, 108]]),
                    vap(addc[:], 0, [[0, RT], [1, 108]]), add)

                # ---- softmax (unnormalized exp -> normalize) ---------------
                el = wpool.tile([C, RT * 36], f32, name="el")
                nc.scalar.activation(
                    vap(el[:], 0, [[36, RT], [1, 36]]),
                    vap(rawq[:], 72, [[108, RT], [1, 36]]), AF.Exp)
                den = wpool.tile([C, RT * G], f32, name="den")
                nc.vector.tensor_reduce(
                    vap(den[:], 0, [[G, RT], [1, G]]),
                    vap(el[:], 0, [[36, RT], [9, G], [1, P]]),
                    mybir.AxisListType.X, add)
                denr = wpool.tile([C, RT * G], f32, name="denr")
                nc.vector.reciprocal(denr[:], den[:])
                nc.vector.tensor_tensor(
                    vap(el[:], 0, [[36, RT], [9, G], [1, P]]),
                    vap(el[:], 0, [[36, RT], [9, G], [1, P]]),
                    vap(denr[:], 0, [[G, RT], [1, G], [0, P]]), mult)

                # ---- hat weights ------------------------------------------
                hats = []
                for i in range(NTAP):
                    habs = wpool.tile([C, RT * 72], f32, name=f"habs{i}")
                    nc.scalar.activation(
                        vap(habs[:], 0, [[72, RT], [1, 72]]),
                        vap(rawq[:], 0, [[108, RT], [1, 72]]),
                        AF.Abs, bias=cvals[:, i:i + 1])
                    h = wpool.tile([C, RT * 72], f32, name=f"hat{i}")
                    nc.scalar.activation(h[:], habs[:], AF.Relu,
                                         bias=cvals[:, 4:5], scale=-1.0)
                    hats.append(h)

                # ---- per-cell coefficients --------------------------------
                mh = []
                for i in range(NTAP):
                    mt = wpool.tile([C, RT * 36], f32, name=f"mh{i}")
                    nc.vector.tensor_tensor(
                        vap(mt[:], 0, [[36, RT], [1, 36]]),
                        vap(el[:], 0, [[36, RT], [1, 36]]),
                        vap(hats[i][:], 0, [[72, RT], [1, 36]]), mult)
                    mh.append(mt)
                coef = wpool.tile([C, NTAP * NTAP * RT * 36], f32, name="coef")
                for i in range(NTAP):
                    for j in range(NTAP):
                        s = i * NTAP + j
                        nc.vector.tensor_tensor(
                            vap(coef[:], s * RT * 36, [[36, RT], [1, 36]]),
                            vap(mh[i][:], 0, [[36, RT], [1, 36]]),
                            vap(hats[j][:], 36, [[72, RT], [1, 36]]), mult)

                # ---- apply ------------------------------------------------
                acc = wpool.tile([C, RT * C], f32, name="acc")
                for r in range(RT):
                    for gg in range(G):
                        first = True
                        aslice = vap(acc[:], r * C + gg * gc, [[1, gc]])
                        for p in range(P):
                            by, bx = int(g.By[gg, p]), int(g.Bx[gg, p])
                            for i in range(int(g.spy[gg, p])):
                                rho = r + 2 + by + i
                                for j in range(int(g.spx[gg, p])):
                                    si = bx + j - g.SMIN
                                    xv = vap(xs[:], (si * NR + rho) * C + gg * gc, [[1, gc]])
                                    cidx = (i * NTAP + j) * RT * 36 + r * 36 + gg * P + p
                                    ccol = vap(coef[:], cidx, [[1, 1]])
                                    if first:
                                        nc.vector.tensor_scalar_mul(aslice, xv, ccol)
                                        first = False
                                    else:
                                        nc.vector.scalar_tensor_tensor(
                                            aslice, xv, ccol, aslice, mult, add)

                nc.sync.dma_start(
                    bass.AP(tensor=out_t, offset=RT * t * W * C,
                            ap=[[C, W], [W * C, RT], [1, C]]),
                    vap(acc[:], 0, [[C, RT], [1, C]]))

    nc.compile()
    return nc


def _host_prep(inp, W_off, b_off, W_mask, b_mask, g):
    xhw = inp.reshape(B, H, W, C)

    wcat = np.empty((C, 108), np.float32)
    addc_row = np.empty(108, np.float32)
    for gg in range(G):
        for p in range(P):
            gp = gg * P + p
            wcat[:, gp] = W_off[:, 2 * gp + 1]           # y
            wcat[:, 36 + gp] = W_off[:, 2 * gp]          # x
            wcat[:, 72 + gp] = W_mask[:, gp]
            addc_row[gp] = b_off[2 * gp + 1] + (KY[p] - g.By[gg, p])
            addc_row[36 + gp] = b_off[2 * gp] + (KX[p] - g.Bx[gg, p])
            addc_row[72 + gp] = b_mask[gp]
    addc = np.tile(addc_row[None, :], (C, 1))
    ident = np.eye(C, dtype=np.float32)
    cvals = np.zeros((C, 8), np.float32)
    for i in range(NTAP):
        cvals[:, i] = -float(i)
    cvals[:, 4] = 1.0

    in_maps = []
    for core in range(N_CORES):
        b, half = divmod(core, 2)
        h0 = HS * half
        # slab rows: padded rows [h0-1, h0-1+NROW) ; cols: padded [-2, NCOL-2)
        xslab = np.zeros((g.NROW, g.NCOL, C), np.float32)
        for lr in range(g.NROW):
            orig = lr + h0 - 2
            if 0 <= orig < H:
                xslab[lr, 3:3 + W, :] = xhw[b, orig]
        xchw = np.ascontiguousarray(
            xhw[b, h0:h0 + HS].reshape(HS * W, C).T)
        in_maps.append({
            "xslab": xslab.reshape(-1),
            "xchw": xchw,
            "wcat": wcat,
            "addc": addc,
            "ident": ident,
            "cvals": cvals,
        })
    return in_maps


def _make_geom(inp, W_off, b_off):
    g = _Geom()
    g.Bx, g.By, g.spx, g.spy = _geometry(inp, W_off, b_off)
    g.SMIN = int(g.Bx.min())
    smax = int((g.Bx + g.spx - 1).max())
    g.NS = smax - g.SMIN + 1
    rmin = int(2 + g.By.min())            # rho = r+2+By+i ; r=0,i=0
    rmax = int(RT - 1 + 2 + (g.By + g.spy - 1).max())
    assert rmin >= 0
    g.NR = rmax + 1
    g.NROW = RT * (NTILES - 1) + g.NR     # slab rows per core
    # slab col for (wo, si): wo + si + (3 + SMIN) ; worst col = 127+NS-1+3+SMIN
    g.C0 = 3 + g.SMIN                     # col offset baked into slab layout
    g.NCOL = W + g.NS - 1 + g.C0 + 1
    return g


def _run(inp, W_off, b_off, W_mask, b_mask, **spmd_kwargs):
    inp = np.ascontiguousarray(inp, np.float32)
    g = _make_geom(inp, np.asarray(W_off, np.float32), np.asarray(b_off, np.float32))
    nc = _build(g)
    in_maps = _host_prep(inp, np.asarray(W_off, np.float32),
                         np.asarray(b_off, np.float32),
                         np.asarray(W_mask, np.float32),
                         np.asarray(b_mask, np.float32), g)
    res = run_bass_kernel_spmd(nc, in_maps, core_ids=list(range(N_CORES)),
                               **spmd_kwargs)
    out = np.empty((B, H, W, C), np.float32)
    for core in range(N_CORES):
        b, half = divmod(core, 2)
        out[b, HS * half:HS * (half + 1)] = \
            res.results[core]["out"].reshape(HS, W, C)
    return out.reshape(B, C, H, W), res


def kernel(inp, W_off, b_off, W_mask, b_mask):
    out, _ = _run(inp, W_off, b_off, W_mask, b_mask)
    return out


if __name__ == "__main__":
    d = np.load("/root/problem/ref_cache.npz")
    got = kernel(d["inp"], d["W_off"], d["b_off"], d["W_mask"], d["b_mask"])
    exp = d["exp"]
    err = np.abs(got - exp).max()
    print("absmax err:", err, "rel:", err / np.abs(exp).max())



# revision 13
# speedup vs baseline: 6.3262x; 6.3262x over previous
"""DCNv3 forward on 8 trn2 NeuronCores.

Strategy (data-parallel over (batch, H-half) -> 8 shards), v2:

The v1 kernel was DVE-bound: 20K tiny (FD=32) scalar_tensor_tensor ops
applying per-(g,p,i,j) tap coefficients pixel-column by pixel-column.
v2 restructures the whole coefficient pipeline into a
"taps-on-partitions, pixels-on-free" layout so every elementwise op has
FD=512, and uses the PE for all cross-partition data movement:

  - praw matmuls produce per-TAP duplicated offset rows directly
    (weight matrix columns are duplicated per tap on the host)
  - hat weights: 2 ACT ops per chunk (bias folds the per-tap integer)
  - softmax: exp on ACT, group-sum + reciprocal-replicate via tiny
    0/1-indicator matmuls on PE
  - tap coefficients el*haty*hatx: 2 big TT ops per chunk (fp16, 2x)
  - tap -> (cell,g) collapse: one PE matmul with a constant 0/1 matrix
    (cells = distinct (dy,s) integer sample shifts; 321 taps -> 103
    live (g,cell) pairs packed into 128 rows = 32 cells x 4 groups)
  - PE transpose moves Band to [pixel-column, (row, cell)] layout
  - apply: per cell, ONE fp16 TT mult (coef broadcast over channels)
    + ONE fp16 TT add over all RT rows x contiguous-group channels
    (~72 ops of FD~=2-8K per 16-row tile instead of ~5100 FD=32 ops)

Everything runs in fp16 (DVE 2x mode where APs allow); accumulation
inside matmuls is fp32 (PSUM).
"""

import numpy as np
import sys

sys.path.insert(0, "/opt/trn_rl_repo")

import concourse.bass as bass
import concourse.bacc as bacc
import concourse.mybir as mybir
import concourse.tile as tile
from concourse.bass_utils import run_bass_kernel_spmd

B, C, H, W = 4, 128, 128, 128
G, P, gc = 4, 9, 32
N_CORES = 8
HS = H // 2          # rows per core (b, half)
RT = 16              # output rows per device tile
NTILES = HS // RT    # 4
NQ = RT * W // 512   # 512-pixel chunks per tile (PSUM bank = 512 fp32)

f32 = mybir.dt.float32
f16 = mybir.dt.float16

_KS = np.array([-1.0, 0.0, 1.0], np.float32)
KX = np.repeat(_KS, 3)   # x-major flatten (matches torch meshgrid in ref)
KY = np.tile(_KS, 3)


class _Geom:
    pass


def _make_geom(inp, W_off, b_off):
    """Integer tap geometry from the actual offset field (host-side)."""
    g = _Geom()
    xhw = inp.reshape(B, H, W, C)
    off = (xhw.reshape(-1, C) @ W_off + b_off).reshape(-1, G, P, 2)
    rx = off[..., 0] + KX
    ry = off[..., 1] + KY
    g.Bx = np.floor(rx.min(axis=0)).astype(np.int64)
    g.By = np.floor(ry.min(axis=0)).astype(np.int64)
    g.spx = np.minimum(np.floor(rx.max(axis=0)).astype(np.int64) + 2 - g.Bx, 4)
    g.spy = np.minimum(np.floor(ry.max(axis=0)).astype(np.int64) + 2 - g.By, 4)

    g.taps = [(gg, p, i, j) for gg in range(G) for p in range(P)
              for i in range(int(g.spy[gg, p])) for j in range(int(g.spx[gg, p]))]
    g.T = len(g.taps)
    cells = sorted({(int(g.By[gg, p]) + i, int(g.Bx[gg, p]) + j)
                    for (gg, p, i, j) in g.taps})
    assert len(cells) <= 32, f"too many cells: {len(cells)}"
    g.cells = cells
    g.NCELL = len(cells)
    cidx = {c: k for k, c in enumerate(cells)}
    g.kappa = [cidx[(int(g.By[gg, p]) + i, int(g.Bx[gg, p]) + j)] * 4 + gg
               for (gg, p, i, j) in g.taps]
    # group-contiguous apply runs per cell: (cellidx, dy, s, g0, ng)
    livemask = set(g.kappa)
    g.runs = []
    for k, (dy, s) in enumerate(cells):
        gg = 0
        while gg < G:
            if k * 4 + gg in livemask:
                g0 = gg
                while gg < G and k * 4 + gg in livemask:
                    gg += 1
                g.runs.append((k, dy, s, g0, gg - g0))
            else:
                gg += 1

    g.dymin = min(c[0] for c in cells)
    g.dymax = max(c[0] for c in cells)
    g.smin = min(c[1] for c in cells)
    g.smax = max(c[1] for c in cells)
    g.NS = g.smax - g.smin + 1
    g.C0 = 3 + g.smin            # slab col = wo + (s - smin) + C0 + ... = wo+s+3
    assert g.C0 >= 0
    g.NR = RT + (g.dymax - g.dymin)  # slab rows per tile window
    g.NROW = RT * (NTILES - 1) + g.NR + 1
    g.NCOL = W + g.NS - 1 + g.C0 + 1
    # tap chunks of <=128 rows
    g.nch = (g.T + 127) // 128
    g.csz = [(g.T + g.nch - 1) // g.nch] * g.nch
    g.csz[-1] = g.T - sum(g.csz[:-1])
    g.cof = [sum(g.csz[:c]) for c in range(g.nch)]
    return g


def _build(g: "_Geom"):
    nc = bacc.Bacc("TRN2", target_bir_lowering=False, debug=False,
                   num_devices=N_CORES)

    xslab_t = nc.dram_tensor("xslab", [g.NROW * g.NCOL * C], f16, kind="ExternalInput")
    xchw_t = nc.dram_tensor("xchw", [C, HS * W], f16, kind="ExternalInput")
    # packed constant blobs (fp16 matrices + fp32 bias columns)
    wy_t = nc.dram_tensor("wy", [C, g.nch * 128], f16, kind="ExternalInput")
    wx_t = nc.dram_tensor("wx", [C, g.nch * 128], f16, kind="ExternalInput")
    wlog_t = nc.dram_tensor("wlog", [C, G * P], f16, kind="ExternalInput")
    indrep_t = nc.dram_tensor("indrep", [G * P, g.nch * 128], f16, kind="ExternalInput")
    indden_t = nc.dram_tensor("indden", [G * P, G], f16, kind="ExternalInput")
    indr36_t = nc.dram_tensor("indr36", [G, G * P], f32, kind="ExternalInput")
    coll_t = nc.dram_tensor("coll", [128, g.nch * 128], f16, kind="ExternalInput")
    ident_t = nc.dram_tensor("ident", [128, 128], f16, kind="ExternalInput")
    bias_t = nc.dram_tensor("bias", [128, 2 * g.nch + 2], f32, kind="ExternalInput")
    out_t = nc.dram_tensor("out", [HS * W * C], f16, kind="ExternalOutput")

    NS, NR, nch = g.NS, g.NR, g.nch
    mult, add = mybir.AluOpType.mult, mybir.AluOpType.add
    AF = mybir.ActivationFunctionType

    def vap(v, off, dims):
        return bass.AP(tensor=v.tensor, offset=v.offset + off, ap=[v.ap[0]] + dims)

    with tile.TileContext(nc) as tc:
        with (
            tc.tile_pool(name="const", bufs=1) as cpool,
            tc.tile_pool(name="xs", bufs=2) as xspool,
            tc.tile_pool(name="work", bufs=2) as wpool,
            tc.tile_pool(name="psmm", bufs=2, space="PSUM") as pmm,
            tc.tile_pool(name="pssm", bufs=1, space="PSUM") as psm,
            tc.tile_pool(name="psband", bufs=1, space="PSUM") as pband,
            tc.tile_pool(name="pstr", bufs=2, space="PSUM") as ptr,
        ):
            # ---- constants: DMA land, then ACT copy (matmul single-sem rule)
            def cload(name, shape, dt, src_ap):
                t0 = cpool.tile(shape, dt, name=name + "0")
                nc.sync.dma_start(t0[:], src_ap)
                t1 = cpool.tile(shape, dt, name=name)
                nc.scalar.copy(t1[:], t0[:])
                return t1

            wy = cload("wy", [C, nch * 128], f16, wy_t.ap())
            wx = cload("wx", [C, nch * 128], f16, wx_t.ap())
            wlog = cload("wlog", [C, G * P], f16, wlog_t.ap())
            indrep = cload("indrep", [G * P, nch * 128], f16, indrep_t.ap())
            indden = cload("indden", [G * P, G], f16, indden_t.ap())
            indr36 = cload("indr36", [G, G * P], f32, indr36_t.ap())
            coll = cload("coll", [128, nch * 128], f16, coll_t.ap())
            ident = cload("ident", [128, 128], f16, ident_t.ap())
            biasc = cpool.tile([128, 2 * nch + 2], f32, name="biasc")
            nc.sync.dma_start(biasc[:], bias_t.ap())

            for t in range(NTILES):
                # ---- loads ----------------------------------------------
                xs = xspool.tile([C, NS * NR * C], f16, name="xs")
                for si in range(NS):
                    src = bass.AP(
                        tensor=xslab_t,
                        offset=(RT * t * g.NCOL + g.C0 + si) * C,
                        ap=[[C, W], [g.NCOL * C, NR], [1, C]])
                    nc.sync.dma_start(
                        vap(xs[:], si * NR * C, [[C, NR], [1, C]]), src)

                xc0 = wpool.tile([C, RT * W], f16, name="xc0")
                nc.sync.dma_start(
                    xc0[:], bass.AP(tensor=xchw_t, offset=RT * t * W,
                                    ap=[[HS * W, C], [1, RT * W]]))
                xc = wpool.tile([C, RT * W], f16, name="xc")
                nc.scalar.copy(xc[:], xc0[:])

                bandT = wpool.tile([C, RT * 128], f16, name="bandT")

                for q in range(NQ):
                    xcq = xc[:, q * 512:(q + 1) * 512]

                    # softmax over P, normalized el rows ------------------
                    lg_ps = psm.tile([G * P, 512], f32, name="lg")
                    nc.tensor.matmul(lg_ps[:], wlog[:], xcq, start=True, stop=True)
                    el36 = wpool.tile([G * P, 512], f16, name="el36")
                    nc.scalar.activation(el36[:], lg_ps[:], AF.Exp,
                                         bias=biasc[:G * P, 2 * nch:2 * nch + 1])
                    den_ps = psm.tile([G, 512], f32, name="den")
                    nc.tensor.matmul(den_ps[:], indden[:], el36[:],
                                     start=True, stop=True)
                    denr = wpool.tile([G, 512], f32, name="denr")
                    nc.vector.reciprocal(denr[:], den_ps[:])
                    d36_ps = psm.tile([G * P, 512], f32, name="d36")
                    nc.tensor.matmul(d36_ps[:], indr36[:], denr[:],
                                     start=True, stop=True)
                    d36 = wpool.tile([G * P, 512], f16, name="d36s")
                    nc.scalar.copy(d36[:], d36_ps[:])
                    el36n = wpool.tile([G * P, 512], f16, name="el36n")
                    nc.vector.tensor_tensor(el36n[:], el36[:], d36[:], mult)

                    # per-chunk: offsets -> hats -> tap coefs -> collapse -
                    band_ps = pband.tile([128, 512], f32, name="band")
                    for ch in range(nch):
                        m = g.csz[ch]
                        oy_ps = pmm.tile([128, 512], f32, name="mm")
                        nc.tensor.matmul(oy_ps[:m], wy[:, ch * 128:ch * 128 + m],
                                         xcq, start=True, stop=True)
                        hy = wpool.tile([128, 512], f16, name="hy")
                        nc.scalar.activation(hy[:m], oy_ps[:m], AF.Abs,
                                             bias=biasc[:m, ch:ch + 1])
                        nc.scalar.activation(hy[:m], hy[:m], AF.Relu,
                                             bias=biasc[:m, 2 * nch + 1:2 * nch + 2],
                                             scale=-1.0)
                        ox_ps = pmm.tile([128, 512], f32, name="mm")
                        nc.tensor.matmul(ox_ps[:m], wx[:, ch * 128:ch * 128 + m],
                                         xcq, start=True, stop=True)
                        hx = wpool.tile([128, 512], f16, name="hx")
                        nc.scalar.activation(hx[:m], ox_ps[:m], AF.Abs,
                                             bias=biasc[:m, nch + ch:nch + ch + 1])
                        nc.scalar.activation(hx[:m], hx[:m], AF.Relu,
                                             bias=biasc[:m, 2 * nch + 1:2 * nch + 2],
                                             scale=-1.0)
                        er_ps = pmm.tile([128, 512], f32, name="mm")
                        nc.tensor.matmul(er_ps[:m],
                                         indrep[:, ch * 128:ch * 128 + m],
                                         el36n[:], start=True, stop=True)
                        er = wpool.tile([128, 512], f16, name="er")
                        nc.scalar.copy(er[:m], er_ps[:m])
                        tp = wpool.tile([128, 512], f16, name="tp")
                        nc.vector.tensor_tensor(tp[:m], hy[:m], hx[:m], mult)
                        nc.vector.tensor_tensor(tp[:m], tp[:m], er[:m], mult)
                        nc.tensor.matmul(band_ps[:],
                                         coll[:m, ch * 128:(ch + 1) * 128],
                                         tp[:m], start=(ch == 0),
                                         stop=(ch == nch - 1))

                    band_sb = wpool.tile([128, 512], f16, name="bandsb")
                    nc.scalar.copy(band_sb[:], band_ps[:])
                    # transpose to [wo, cell] per output row --------------
                    for r in range(4):
                        tr_ps = ptr.tile([128, 128], f16, name="tr")
                        nc.tensor.transpose(tr_ps[:],
                                            band_sb[:, r * 128:(r + 1) * 128],
                                            ident[:])
                        rr = q * 4 + r
                        nc.scalar.copy(
                            vap(bandT[:], rr * 128, [[1, 128]]), tr_ps[:])

                # ---- apply ----------------------------------------------
                acc = wpool.tile([C, RT * C], f16, name="acc")
                nc.vector.memset(acc[:], 0)
                tmp = wpool.tile([C, RT * C], f16, name="tmp")
                for (k, dy, s, g0, ng) in g.runs:
                    xoff = (((s - g.smin) * NR + (dy - g.dymin)) * C + g0 * gc)
                    xv = vap(xs[:], xoff, [[C, RT], [1, gc * ng]])
                    cf = vap(bandT[:], k * 4 + g0, [[128, RT], [1, ng], [0, gc]])
                    tv = vap(tmp[:], g0 * gc, [[C, RT], [1, gc * ng]])
                    av = vap(acc[:], g0 * gc, [[C, RT], [1, gc * ng]])
                    nc.vector.tensor_tensor(tv, xv, cf, mult)
                    nc.vector.tensor_tensor(av, tv, av, add)

                nc.sync.dma_start(
                    bass.AP(tensor=out_t, offset=RT * t * W * C,
                            ap=[[C, W], [W * C, RT], [1, C]]),
                    vap(acc[:], 0, [[C, RT], [1, C]]))

    nc.compile()
    return nc


def _host_prep(inp, W_off, b_off, W_mask, b_mask, g):
    xhw = inp.reshape(B, H, W, C)
    nch, T = g.nch, g.T

    wy = np.zeros((C, nch * 128), np.float16)
    wx = np.zeros((C, nch * 128), np.float16)
    bias = np.zeros((128, 2 * nch + 2), np.float32)
    indrep = np.zeros((G * P, nch * 128), np.float16)
    coll = np.zeros((128, nch * 128), np.float16)
    for t, (gg, p, i, j) in enumerate(g.taps):
        ch, rr = t // g.csz[0], t % g.csz[0]
        gp = gg * P + p
        wy[:, ch * 128 + rr] = W_off[:, 2 * gp + 1]
        wx[:, ch * 128 + rr] = W_off[:, 2 * gp]
        bias[rr, ch] = b_off[2 * gp + 1] + KY[p] - (g.By[gg, p] + i)
        bias[rr, nch + ch] = b_off[2 * gp] + KX[p] - (g.Bx[gg, p] + j)
        indrep[gp, ch * 128 + rr] = 1.0
        coll[rr, ch * 128 + g.kappa[t]] = 1.0
    bias[:G * P, 2 * nch] = b_mask
    bias[:, 2 * nch + 1] = 1.0
    wlog = W_mask.astype(np.float16)
    indden = np.zeros((G * P, G), np.float16)
    for gg in range(G):
        indden[gg * P:(gg + 1) * P, gg] = 1.0
    indr36 = np.zeros((G, G * P), np.float32)
    for gg in range(G):
        indr36[gg, gg * P:(gg + 1) * P] = 1.0
    ident = np.eye(128, dtype=np.float16)

    in_maps = []
    for core in range(N_CORES):
        b, half = divmod(core, 2)
        h0 = HS * half
        xslab = np.zeros((g.NROW, g.NCOL, C), np.float16)
        for lr in range(g.NROW):
            orig = lr + h0 + g.dymin
            if 0 <= orig < H:
                xslab[lr, 3:3 + W, :] = xhw[b, orig]
        xchw = np.ascontiguousarray(
            xhw[b, h0:h0 + HS].reshape(HS * W, C).T).astype(np.float16)
        in_maps.append({
            "xslab": xslab.reshape(-1),
            "xchw": xchw,
            "wy": wy, "wx": wx, "wlog": wlog,
            "indrep": indrep, "indden": indden, "indr36": indr36,
            "coll": coll, "ident": ident, "bias": bias,
        })
    return in_maps


def _run(inp, W_off, b_off, W_mask, b_mask, **spmd_kwargs):
    inp = np.ascontiguousarray(inp, np.float32)
    W_off = np.asarray(W_off, np.float32)
    b_off = np.asarray(b_off, np.float32)
    g = _make_geom(inp, W_off, b_off)
    nc = _build(g)
    in_maps = _host_prep(inp, W_off, b_off,
                         np.asarray(W_mask, np.float32),
                         np.asarray(b_mask, np.float32), g)
    res = run_bass_kernel_spmd(nc, in_maps, core_ids=list(range(N_CORES)),
                               **spmd_kwargs)
    out = np.empty((B, H, W, C), np.float32)
    for core in range(N_CORES):
        b, half = divmod(core, 2)
        out[b, HS * half:HS * (half + 1)] = \
            res.results[core]["out"].astype(np.float32).reshape(HS, W, C)
    return out.reshape(B, C, H, W), res


def kernel(inp, W_off, b_off, W_mask, b_mask):
    out, _ = _run(inp, W_off, b_off, W_mask, b_mask)
    return out


if __name__ == "__main__":
    d = np.load("/root/problem/ref_cache.npz")
    got = kernel(d["inp"], d["W_off"], d["b_off"], d["W_mask"], d["b_mask"])
    exp = d["exp"]
    err = np.abs(got - exp).max()
    print("absmax err:", err, "rel:", err / np.abs(exp).max())
